# revision 16
# baseline (speedup 1.0000x reference)
"""Trainium2 Bass kernel for nn_ConvexReLUCNN.

Math (identical multilinear form as the reference, reordered):
    reference:  U = unfold(x,3); A = U.G^T (54 GFLOP); out = A.(v-w)
    here:       C[ko, n] = sum_m (v-w)[m, ko] * G[m, n]   (~1 GFLOP, once)
                Wmat[chw, o] = fold_3x3(C)                (tiny shift-adds)
                out = x_flat @ Wmat                       (~0.13 GFLOP)

Distribution: sharded by image row band. Core i owns output-image rows
h in [8i, 8i+8) (all channels, all widths, ALL batches):
  - x shard (pre-transposed on host to [chw, b], bf16): (1536, 512)
  - G shard: image rows in [8i-2, 8i+8) of G's 62x62 patch grid,
    zero-padded at the edges to a uniform 10 rows, bf16   (512, 620)
  - pd = v - w computed on host, permuted/padded to the matmul layout
    ko'' = pq*32 + c*10 + o, bf16                         (512, 288)
Each core computes partial out^T (10, 512) over its chw band; the host
sums the 8 partials and transposes - no device collectives needed.

All layout work (transpose/interleave/zero-pad/cast) is host-side
sharding; all contractions run on device.
"""

import numpy as np
from contextlib import ExitStack

import ml_dtypes

import concourse.bass as bass
import concourse.mybir as mybir
import concourse.tile as tile
from concourse import bacc
from concourse.bass_utils import run_bass_kernel_spmd
from concourse.masks import make_identity

N_CORES = 8
B_FULL = 512
C_CH, H, W = 3, 64, 64
HB = H // N_CORES           # 8 image rows per core
BAND = C_CH * HB * W        # 1536 chw positions per core
M = 512                     # num_neurons
KK = 27
O = 10
Ho = Wo = 62
IW = HB + 2                 # 10 patch-grid rows feeding one band
NL = IW * Wo                # 620 local G columns
Z = 32                      # padded (c,o) block per (p,q): 3*10 -> 32
KO2 = 9 * Z                 # 288

F32 = mybir.dt.float32
F16 = mybir.dt.float16
BF16 = mybir.dt.bfloat16
NP_BF16 = ml_dtypes.bfloat16

_NC = None


def _build():
    nc = bacc.Bacc("TRN2", target_bir_lowering=False, debug=False,
                   num_devices=N_CORES)
    # host supplies all operands pre-interleaved for straight 128-partition
    # DMAs: dim0 = partition.
    x_d = nc.dram_tensor("x", [128, 12, 512], BF16, kind="ExternalInput").ap()
    g_d = nc.dram_tensor("G", [128, 4, NL], BF16, kind="ExternalInput").ap()
    pd_d = nc.dram_tensor("pd", [128, 4, KO2], BF16,
                          kind="ExternalInput").ap()
    o_d = nc.dram_tensor("out", [O, B_FULL], F32, kind="ExternalOutput").ap()

    with tile.TileContext(nc) as tc, ExitStack() as ctx:
        const = ctx.enter_context(tc.tile_pool(name="const", bufs=1))
        big = ctx.enter_context(tc.tile_pool(name="big", bufs=1))
        psC = ctx.enter_context(tc.tile_pool(name="psC", bufs=2, space="PSUM"))
        psT = ctx.enter_context(tc.tile_pool(name="psT", bufs=2, space="PSUM"))
        psF = ctx.enter_context(tc.tile_pool(name="psF", bufs=1, space="PSUM"))
        stage = ctx.enter_context(tc.tile_pool(name="stage", bufs=2))

        # ---- loads first (nothing may delay DMA issue); G/pd split per
        # m-tile so the first C matmuls start as soon as tile 0 lands ------
        gs = big.tile([128, 4, NL], BF16, tag="gs")
        pds = big.tile([128, 4, KO2], BF16, tag="pds")
        xs = big.tile([128, 12, 512], BF16, tag="xs")
        for m in range(4):
            nc.sync.dma_start(gs[:, m], g_d[:, m])
            nc.sync.dma_start(pds[:, m], pd_d[:, m])
        # x must not compete with G/pd for HBM bandwidth (the C matmuls are
        # gated on those); chain its DMA behind the last G tile via a dummy
        # 1-element copy into xs.
        nc.vector.tensor_copy(xs[0:1, 0, 0:1], gs[0:1, 3, 0:1])
        nc.sync.dma_start(xs[:], x_d)

        ident = const.tile([128, 128], F16)
        make_identity(nc, ident[:])
        Wacc = big.tile([128, HB * W], F16, tag="Wacc")
        nc.gpsimd.memset(Wacc[:], 0.0)

        # ---- C[ko'', n_local] = pd2.T @ G_shard --------------------------
        # rows [0,128) = pq 0-3, [128,256) = pq 4-7, [256,288) = pq 8,
        # quadrant base 32*(pq%4) within each tile. Each chunk's C stays in
        # a bank-spanning [128, 620] PSUM tile; the fold reads PSUM
        # directly (no SBUF staging, no scalar engine - its ACT_TABLE_LOAD
        # would stall the whole kernel start by ~2.5us).
        KO_CH = [(0, 0, 128), (1, 128, 128), (2, 256, 32)]
        PIECES = [(0, 512), (512, NL - 512)]
        CPS = []

        def c_chunk(ci):
            ko0, kow = KO_CH[ci][1], KO_CH[ci][2]
            ps = psC.tile([128, NL], F32, tag="psC")
            CPS.append(ps)
            for (p0, pw) in PIECES:
                for m in range(4):
                    nc.tensor.matmul(
                        ps[:kow, p0:p0 + pw],
                        pds[:, m, ko0:ko0 + kow],
                        gs[:, m, p0:p0 + pw],
                        start=(m == 0), stop=(m == 3))

        # fold: Wmat[c, 8i+dh, w, o] = sum_pq C[(c,p,q,o), (dh+2-p, w-q)]
        Wv = Wacc[:].rearrange("p (h w) -> p h w", w=W)

        def fold(pq):
            p_, q_ = divmod(pq, 3)
            ti, qd = pq // 4, pq % 4
            base = 32 * qd
            Cv = CPS[ti][base:base + 32, :].rearrange(
                "p (i j) -> p i j", j=Wo)
            src = Cv[:, 2 - p_:2 - p_ + HB, :]
            dst = Wv[base:base + 32, :, q_:q_ + Wo]
            nc.vector.tensor_add(dst, dst, src)

        # psC has 2 bufs: chunk2 reuses chunk0's banks, so chunk0's folds
        # are issued before chunk2's matmuls (they run under chunk1's).
        c_chunk(0)
        c_chunk(1)
        for pq in (0, 1, 2, 3):
            fold(pq)
        c_chunk(2)
        for pq in (4, 5, 6, 7, 8):
            fold(pq)

        # ---- transpose Wacc chunks, combine quadrants, cast to bf16 ------
        WsB = big.tile([128, 4, Z], BF16, tag="WsB")
        for j in range(4):
            pst = psT.tile([128, 128], F16, tag="psT")
            nc.tensor.transpose(pst[:], Wacc[:, 128 * j:128 * (j + 1)],
                                ident[:])
            t0 = stage.tile([128, Z], F16, tag="t0")
            nc.vector.tensor_copy(t0[:], pst[:, 0:32])
            nc.vector.tensor_add(t0[:], t0[:], pst[:, 32:64])
            nc.vector.tensor_add(t0[:], t0[:], pst[:, 64:96])
            nc.vector.tensor_add(WsB[:, j, :], t0[:], pst[:, 96:128])

        # ---- final: partial out^T[o, b] over this core's 12 chw chunks ---
        pf = psF.tile([O, B_FULL], F32, tag="pf")
        n = 0
        for j in range(4):
            for c in range(3):
                nc.tensor.matmul(pf[:, :],
                                 WsB[:, j, c * O:(c + 1) * O],
                                 xs[:, c * 4 + j, :],
                                 start=(n == 0), stop=(n == 11))
                n += 1
        obuf = stage.tile([O, B_FULL], F32, tag="obuf")
        nc.vector.tensor_copy(obuf[:], pf[:])
        nc.sync.dma_start(o_d, obuf[:])
    nc.compile()
    return nc


def _get_nc():
    global _NC
    if _NC is None:
        _NC = _build()
    return _NC


def _shard_inputs(inputs):
    x = np.ascontiguousarray(inputs["x"], dtype=np.float32)   # (512,3,64,64)
    G = np.ascontiguousarray(inputs["G"], dtype=np.float32)   # (512,3844)
    v = np.ascontiguousarray(inputs["v"], dtype=np.float32)
    w = np.ascontiguousarray(inputs["w"], dtype=np.float32)

    # pd permuted to ko'' = pq*32 + c*10 + o, zero-padded, m-interleaved
    pd = (v - w).reshape(M, C_CH, 9, O)                       # (m, c, pq, o)
    pd2 = np.zeros((M, 9, Z), np.float32)
    for c in range(C_CH):
        pd2[:, :, c * O:(c + 1) * O] = pd[:, c]
    pdh = np.ascontiguousarray(
        pd2.reshape(4, 128, KO2).transpose(1, 0, 2)).astype(NP_BF16)

    Gim = G.reshape(M, Ho, Wo)
    in_maps = []
    for i in range(N_CORES):
        h0 = HB * i
        # x band, transposed to [chw, b] then 128-partition interleaved
        xb = x[:, :, h0:h0 + HB, :].transpose(1, 2, 3, 0)     # (c,h,w,b)
        xh = np.ascontiguousarray(
            xb.reshape(12, 128, B_FULL).transpose(1, 0, 2)).astype(NP_BF16)
        # G shard with halo, zero-padded to 10 patch rows, m-interleaved
        gsh = np.zeros((M, IW, Wo), np.float32)
        lo, hi = h0 - 2, h0 + HB
        clo, chi = max(lo, 0), min(hi, Ho)
        gsh[:, clo - lo:chi - lo, :] = Gim[:, clo:chi, :]
        gh = np.ascontiguousarray(
            gsh.reshape(4, 128, NL).transpose(1, 0, 2)).astype(NP_BF16)
        in_maps.append({"x": xh, "G": gh, "pd": pdh})
    return in_maps


def _run(inputs, trace=False, **kw):
    nc = _get_nc()
    in_maps = _shard_inputs(inputs)
    res = run_bass_kernel_spmd(nc, in_maps, list(range(N_CORES)),
                               trace=trace, **kw)
    out = np.zeros((O, B_FULL), np.float64)
    for i in range(N_CORES):
        out += res.results[i]["out"].astype(np.float64)
    return np.ascontiguousarray(out.T).astype(np.float32), res


def kernel(**inputs) -> np.ndarray:
    return _run(inputs)[0]


# revision 17
# speedup vs baseline: 1.0933x; 1.0933x over previous
"""Trainium2 Bass kernel for nn_ConvexReLUCNN.

Math (identical multilinear form as the reference, reordered):
    reference:  U = unfold(x,3); A = U.G^T (54 GFLOP); out = A.(v-w)
    here:       T[(p,z), (i,w)] = sum_{q,m} pd[m,(c,p,q,o)] * Gpad[m, i, w-q]
                (q-shifts realized as rhs column offsets, summed in PSUM)
                Wmat[z, (h,w)]  = sum_p T[(p,z), (h+2-p, w)]   (3 adds)
                out = x_flat @ Wmat                            (~0.13 GFLOP)

Distribution: sharded by image row band. Core i owns output-image rows
h in [8i, 8i+8) (all channels, all widths, ALL batches):
  - x shard (pre-transposed on host to [chw, b], bf16): (1536, 512)
  - G shard: patch-grid rows [8i-2, 8i+8), each row zero-padded 62->64
    with 2 extra leading zero cols so every q/row shift stays in
    bounds, bf16: (512, 642)
  - pd = v - w computed on host, permuted to [q, p*32 + c*10 + o], bf16
Each core computes partial out^T (10, 512) over its chw band; the host
sums the 8 partials and transposes - no device collectives needed.

All layout work (transpose/interleave/zero-pad/cast) is host-side
sharding; all contractions run on device.
"""

import numpy as np
from contextlib import ExitStack

import ml_dtypes

import concourse.bass as bass
import concourse.mybir as mybir
import concourse.tile as tile
from concourse import bacc
from concourse.bass_utils import run_bass_kernel_spmd
from concourse.masks import make_identity

N_CORES = 8
B_FULL = 512
C_CH, H, W = 3, 64, 64
HB = H // N_CORES           # 8 image rows per core
M = 512                     # num_neurons
O = 10
Ho = Wo = 62
IW = HB + 2                 # 10 patch-grid rows feeding one band
GPW = 2 + IW * W            # 642 padded G columns
Z = 32                      # padded (c,o) block per p: 3*10 -> 32
PZ = 3 * Z                  # 96 T rows (p, z)

F32 = mybir.dt.float32
F16 = mybir.dt.float16
BF16 = mybir.dt.bfloat16
NP_BF16 = ml_dtypes.bfloat16

_NC = None


def _build():
    nc = bacc.Bacc("TRN2", target_bir_lowering=False, debug=False,
                   num_devices=N_CORES)
    # host supplies all operands pre-interleaved for straight 128-partition
    # DMAs with large per-partition-contiguous descriptors: dim0 = partition.
    x_d = nc.dram_tensor("x", [128, 12, 512], BF16, kind="ExternalInput").ap()
    g_d = nc.dram_tensor("G", [128, 4, GPW], BF16, kind="ExternalInput").ap()
    pd_d = nc.dram_tensor("pd", [128, 4, 3, PZ], BF16,
                          kind="ExternalInput").ap()
    o_d = nc.dram_tensor("out", [O, B_FULL], F32, kind="ExternalOutput").ap()

    with tile.TileContext(nc) as tc, ExitStack() as ctx:
        const = ctx.enter_context(tc.tile_pool(name="const", bufs=1))
        big = ctx.enter_context(tc.tile_pool(name="big", bufs=1))
        psC = ctx.enter_context(tc.tile_pool(name="psC", bufs=1, space="PSUM"))
        psT = ctx.enter_context(tc.tile_pool(name="psT", bufs=2, space="PSUM"))
        psF = ctx.enter_context(tc.tile_pool(name="psF", bufs=1, space="PSUM"))
        stage = ctx.enter_context(tc.tile_pool(name="stage", bufs=2))

        # ---- loads first (nothing may delay DMA issue) -------------------
        gs = big.tile([128, 4, GPW], BF16, tag="gs")
        pds = big.tile([128, 4, 3, PZ], BF16, tag="pds")
        xs = big.tile([128, 12, 512], BF16, tag="xs")
        nc.sync.dma_start(gs[:], g_d)
        nc.sync.dma_start(pds[:], pd_d)
        nc.sync.dma_start(xs[:], x_d)

        idq = const.tile([Z, Z], F16)
        make_identity(nc, idq[:])

        # ---- T[(p,z), (i,w)] = sum_{q,m} pd.T @ Gpad[(2-q) shifted] ------
        # one [96, 640] PSUM accumulation over all 12 (q, m) pairs; the two
        # column pieces keep each matmul inside one PSUM bank.
        ps = psC.tile([PZ, 644], F32, tag="psC")
        for q in range(3):
            for m in range(4):
                lhs = pds[:, m, q, :]
                first, last = (q == 0 and m == 0), (q == 2 and m == 3)
                nc.tensor.matmul(ps[:, 0:512], lhs,
                                 gs[:, m, 2 - q:2 - q + 512],
                                 start=first, stop=last)
                nc.tensor.matmul(ps[:, 512:640], lhs,
                                 gs[:, m, 2 - q + 512:2 - q + 640],
                                 start=first, stop=last)

        # ---- p-fold: Wacc[z, (h,w)] = sum_p T[(p,z), (h+2-p, w)] ---------
        Wacc = big.tile([Z, HB * W], F16, tag="Wacc")
        nc.vector.tensor_copy(Wacc[:], ps[64:96, 0:512])
        nc.vector.tensor_add(Wacc[:], Wacc[:], ps[32:64, 64:576])
        nc.vector.tensor_add(Wacc[:], Wacc[:], ps[0:32, 128:640])

        # ---- transpose Wacc chunks -> WsB[hw_in_chunk, j, z] -------------
        WsB = big.tile([128, 4, Z], BF16, tag="WsB")
        for j in range(4):
            pst = psT.tile([128, Z], F16, tag="psT")
            nc.tensor.transpose(pst[:], Wacc[:, 128 * j:128 * (j + 1)],
                                idq[:])
            nc.vector.tensor_copy(WsB[:, j, :], pst[:])

        # ---- final: partial out^T[o, b] over this core's 12 chw chunks ---
        pf = psF.tile([O, B_FULL], F32, tag="pf")
        n = 0
        for j in range(4):
            for c in range(3):
                nc.tensor.matmul(pf[:, :],
                                 WsB[:, j, c * O:(c + 1) * O],
                                 xs[:, c * 4 + j, :],
                                 start=(n == 0), stop=(n == 11))
                n += 1
        obuf = stage.tile([O, B_FULL], F32, tag="obuf")
        nc.vector.tensor_copy(obuf[:], pf[:])
        nc.sync.dma_start(o_d, obuf[:])
    nc.compile()
    return nc


def _get_nc():
    global _NC
    if _NC is None:
        _NC = _build()
    return _NC


def _shard_inputs(inputs):
    x = np.ascontiguousarray(inputs["x"], dtype=np.float32)   # (512,3,64,64)
    G = np.ascontiguousarray(inputs["G"], dtype=np.float32)   # (512,3844)
    v = np.ascontiguousarray(inputs["v"], dtype=np.float32)
    w = np.ascontiguousarray(inputs["w"], dtype=np.float32)

    # pd permuted to [q, p*32 + c*10 + o], m-interleaved
    pdfull = (v - w).reshape(M, 3, 3, 3, O)       # [m, c, p, q, o]
    pdq = np.zeros((M, 3, 3, Z), np.float32)      # [m, q, p, z]
    for c in range(C_CH):
        pdq[:, :, :, c * O:(c + 1) * O] = pdfull[:, c].transpose(0, 2, 1, 3)
    pdh = np.ascontiguousarray(
        pdq.reshape(4, 128, 3, PZ).transpose(1, 0, 2, 3)).astype(NP_BF16)

    Gim = G.reshape(M, Ho, Wo)
    in_maps = []
    for i in range(N_CORES):
        h0 = HB * i
        # x band, transposed to [chw, b] then 128-partition interleaved
        xb = x[:, :, h0:h0 + HB, :].transpose(1, 2, 3, 0)     # (c,h,w,b)
        xh = np.ascontiguousarray(
            xb.reshape(12, 128, B_FULL).transpose(1, 0, 2)).astype(NP_BF16)
        # G shard with halo: rows padded 62->64 plus 2 leading zero cols
        gsh = np.zeros((M, IW, Wo), np.float32)
        lo, hi = h0 - 2, h0 + HB
        clo, chi = max(lo, 0), min(hi, Ho)
        gsh[:, clo - lo:chi - lo, :] = Gim[:, clo:chi, :]
        gp = np.zeros((M, GPW), np.float32)
        for r in range(IW):
            gp[:, 2 + r * W:2 + r * W + Wo] = gsh[:, r, :]
        gh = np.ascontiguousarray(
            gp.reshape(4, 128, GPW).transpose(1, 0, 2)).astype(NP_BF16)
        in_maps.append({"x": xh, "G": gh, "pd": pdh})
    return in_maps


def _run(inputs, trace=False, **kw):
    nc = _get_nc()
    in_maps = _shard_inputs(inputs)
    res = run_bass_kernel_spmd(nc, in_maps, list(range(N_CORES)),
                               trace=trace, **kw)
    out = np.zeros((O, B_FULL), np.float64)
    for i in range(N_CORES):
        out += res.results[i]["out"].astype(np.float64)
    return np.ascontiguousarray(out.T).astype(np.float32), res


def kernel(**inputs) -> np.ndarray:
    return _run(inputs)[0]


# revision 20
# speedup vs baseline: 1.2565x; 1.1493x over previous
"""Trainium2 Bass kernel for nn_ConvexReLUCNN.

Math (identical multilinear form as the reference, reordered):
    reference:  U = unfold(x,3); A = U.G^T (54 GFLOP); out = A.(v-w)
    here:       T[(p,z), (i,w)] = sum_{q,m} pd[m,(c,p,q,o)] * Gpad[m, i, w-q]
                (q-shifts realized as rhs column offsets, summed in PSUM)
                Wmat[z, (h,w)]  = sum_p T[(p,z), (h+2-p, w)]   (3 adds)
                out = x_flat @ Wmat                            (~0.13 GFLOP)

Distribution: sharded by image row band. Core i owns output-image rows
h in [8i, 8i+8) (all channels, all widths, ALL batches):
  - x shard (pre-transposed on host to [chw, b], bf16): (1536, 512)
  - G shard: patch-grid rows [8i-2, 8i+8), each row zero-padded 62->64
    with 2 extra leading zero cols so every q/row shift stays in
    bounds, bf16: (512, 642)
  - pd = v - w computed on host, permuted to [q, p*32 + c*10 + o], bf16
Each core computes partial out^T (10, 512) over its chw band; the host
sums the 8 partials and transposes - no device collectives needed.

All layout work (transpose/interleave/zero-pad/cast) is host-side
sharding; all contractions run on device.
"""

import numpy as np
from contextlib import ExitStack

import ml_dtypes

import concourse.bass as bass
import concourse.mybir as mybir
import concourse.tile as tile
from concourse import bacc
from concourse.bass_utils import run_bass_kernel_spmd
from concourse.masks import make_identity

N_CORES = 8
B_FULL = 512
C_CH, H, W = 3, 64, 64
HB = H // N_CORES           # 8 image rows per core
M = 512                     # num_neurons
O = 10
Ho = Wo = 62
IW = HB + 2                 # 10 patch-grid rows feeding one band
GPW = 2 + IW * W            # 642 padded G columns
Z = 32                      # padded (c,o) block per p: 3*10 -> 32
PZ = 3 * Z                  # 96 T rows (p, z)

F32 = mybir.dt.float32
F16 = mybir.dt.float16
BF16 = mybir.dt.bfloat16
NP_BF16 = ml_dtypes.bfloat16

_NC = None


def _build():
    nc = bacc.Bacc("TRN2", target_bir_lowering=False, debug=False,
                   num_devices=N_CORES)
    # host supplies all operands pre-interleaved for straight 128-partition
    # DMAs with large per-partition-contiguous descriptors: dim0 = partition.
    x_d = nc.dram_tensor("x", [128, 12, 512], BF16, kind="ExternalInput").ap()
    g_d = nc.dram_tensor("G", [128, 4, GPW], BF16, kind="ExternalInput").ap()
    pd_d = nc.dram_tensor("pd", [128, 4, 3, PZ], BF16,
                          kind="ExternalInput").ap()
    o_d = nc.dram_tensor("out", [O, B_FULL], F32, kind="ExternalOutput").ap()

    with tile.TileContext(nc) as tc, ExitStack() as ctx:
        const = ctx.enter_context(tc.tile_pool(name="const", bufs=1))
        big = ctx.enter_context(tc.tile_pool(name="big", bufs=1))
        psC = ctx.enter_context(tc.tile_pool(name="psC", bufs=1, space="PSUM"))
        psT = ctx.enter_context(tc.tile_pool(name="psT", bufs=2, space="PSUM"))
        psF = ctx.enter_context(tc.tile_pool(name="psF", bufs=2, space="PSUM"))
        stage = ctx.enter_context(tc.tile_pool(name="stage", bufs=2))

        # ---- loads first (nothing may delay DMA issue) -------------------
        gs = big.tile([128, 4, GPW], BF16, tag="gs")
        pds = big.tile([128, 4, 3, PZ], BF16, tag="pds")
        xs = big.tile([128, 12, 512], BF16, tag="xs")
        nc.sync.dma_start(pds[:], pd_d)
        for m in range(4):
            nc.sync.dma_start(gs[:, m], g_d[:, m])
        nc.sync.dma_start(xs[:], x_d)

        idq = const.tile([Z, Z], F16)
        make_identity(nc, idq[:])

        # ---- T[(p,z), (i,w)] = sum_{q,m} pd.T @ Gpad[(2-q) shifted] ------
        # one [96, 640] PSUM accumulation over all 12 (q, m) pairs; the two
        # column pieces keep each matmul inside one PSUM bank.
        ps = psC.tile([PZ, 644], F32, tag="psC")
        for q in range(3):
            for m in range(4):
                lhs = pds[:, m, q, :]
                first, last = (q == 0 and m == 0), (q == 2 and m == 3)
                nc.tensor.matmul(ps[:, 0:512], lhs,
                                 gs[:, m, 2 - q:2 - q + 512],
                                 start=first, stop=last)
                nc.tensor.matmul(ps[:, 512:640], lhs,
                                 gs[:, m, 2 - q + 512:2 - q + 640],
                                 start=first, stop=last)

        # ---- p-fold: Wacc[z, (h,w)] = sum_p T[(p,z), (h+2-p, w)] ---------
        Wacc = big.tile([Z, HB * W], F16, tag="Wacc")
        nc.vector.tensor_copy(Wacc[:], ps[64:96, 0:512])
        nc.vector.tensor_add(Wacc[:], Wacc[:], ps[32:64, 64:576])
        nc.vector.tensor_add(Wacc[:], Wacc[:], ps[0:32, 128:640])

        # ---- transpose Wacc chunks -> WsB[hw_in_chunk, j, z] -------------
        WsB = big.tile([128, 4, Z], BF16, tag="WsB")
        for j in range(4):
            pst = psT.tile([128, Z], F16, tag="psT")
            nc.tensor.transpose(pst[:], Wacc[:, 128 * j:128 * (j + 1)],
                                idq[:])
            nc.vector.tensor_copy(WsB[:, j, :], pst[:])

        # ---- final: partial out^T[o, b] over this core's 12 chw chunks ---
        # two batch halves so the first half's store overlaps the second
        # half's matmuls.
        for (b0, b1) in ((0, 256), (256, 512)):
            pf = psF.tile([O, 256], F32, tag="pf")
            n = 0
            for j in range(4):
                for c in range(3):
                    nc.tensor.matmul(pf[:, :],
                                     WsB[:, j, c * O:(c + 1) * O],
                                     xs[:, c * 4 + j, b0:b1],
                                     start=(n == 0), stop=(n == 11))
                    n += 1
            obuf = stage.tile([O, 256], F32, tag="obuf")
            nc.vector.tensor_copy(obuf[:], pf[:])
            nc.sync.dma_start(o_d[:, b0:b1], obuf[:])
    nc.compile()
    return nc


def _get_nc():
    global _NC
    if _NC is None:
        _NC = _build()
    return _NC


def _shard_inputs(inputs):
    x = np.ascontiguousarray(inputs["x"], dtype=np.float32)   # (512,3,64,64)
    G = np.ascontiguousarray(inputs["G"], dtype=np.float32)   # (512,3844)
    v = np.ascontiguousarray(inputs["v"], dtype=np.float32)
    w = np.ascontiguousarray(inputs["w"], dtype=np.float32)

    # pd permuted to [q, p*32 + c*10 + o], m-interleaved
    pdfull = (v - w).reshape(M, 3, 3, 3, O)       # [m, c, p, q, o]
    pdq = np.zeros((M, 3, 3, Z), np.float32)      # [m, q, p, z]
    for c in range(C_CH):
        pdq[:, :, :, c * O:(c + 1) * O] = pdfull[:, c].transpose(0, 2, 1, 3)
    pdh = np.ascontiguousarray(
        pdq.reshape(4, 128, 3, PZ).transpose(1, 0, 2, 3)).astype(NP_BF16)

    Gim = G.reshape(M, Ho, Wo)
    in_maps = []
    for i in range(N_CORES):
        h0 = HB * i
        # x band, transposed to [chw, b] then 128-partition interleaved
        xb = x[:, :, h0:h0 + HB, :].transpose(1, 2, 3, 0)     # (c,h,w,b)
        xh = np.ascontiguousarray(
            xb.reshape(12, 128, B_FULL).transpose(1, 0, 2)).astype(NP_BF16)
        # G shard with halo: rows padded 62->64 plus 2 leading zero cols
        gsh = np.zeros((M, IW, Wo), np.float32)
        lo, hi = h0 - 2, h0 + HB
        clo, chi = max(lo, 0), min(hi, Ho)
        gsh[:, clo - lo:chi - lo, :] = Gim[:, clo:chi, :]
        gp = np.zeros((M, GPW), np.float32)
        for r in range(IW):
            gp[:, 2 + r * W:2 + r * W + Wo] = gsh[:, r, :]
        gh = np.ascontiguousarray(
            gp.reshape(4, 128, GPW).transpose(1, 0, 2)).astype(NP_BF16)
        in_maps.append({"x": xh, "G": gh, "pd": pdh})
    return in_maps


def _run(inputs, trace=False, **kw):
    nc = _get_nc()
    in_maps = _shard_inputs(inputs)
    res = run_bass_kernel_spmd(nc, in_maps, list(range(N_CORES)),
                               trace=trace, **kw)
    out = np.zeros((O, B_FULL), np.float64)
    for i in range(N_CORES):
        out += res.results[i]["out"].astype(np.float64)
    return np.ascontiguousarray(out.T).astype(np.float32), res


def kernel(**inputs) -> np.ndarray:
    return _run(inputs)[0]
